# revision 10
# baseline (speedup 1.0000x reference)
"""AdaptiveConv3 Trainium2 kernel.

Full model: 7-layer conv generator (3x3, BN folded on host) -> per-pixel
3x3x6 adaptive kernels (einsum over fixed basis) -> per-pixel contraction
with unfolded input patches.

Sharding: data-parallel over batch N=8, one image per NeuronCore.

Per-core pipeline (image [64, 128, 128]):
  - conv generator on TensorE, channel-major, row-stacked dual buffers so
    vertical tap pairs contract K=128.  Band PAIRS are processed with PE
    array tiling: the two bands' 64-wide outputs go to array column groups
    (0,0)/(0,64) and run concurrently; the K=64 single taps additionally
    row-tile ((0,0) vs (64,64)) by reading band B's rows from the shifted
    bottom-half buffer.
  - the basis contraction is refactored: zb[(l,c)] = x conv basis_l (6
    fixed depthwise 3x3 convs) runs on TensorE as dense matmuls, so the
    per-pixel stage contracts over l=6 instead of the 9 taps
    (out[c,m,p] = sum_l gen[m,l,p] * zb[c,l,p]).
  - per output row, zb and gen are transposed to pixel-major into ONE
    PSUM tile (3x 128-col zbT + 36-col genT) and copied to SBUF with a
    single ScalarE activation.
  - the per-pixel contraction runs on VectorE as 6 custom fused DVE ops
    (one per m): a segmented multiply-accumulate scan over the
    (c-outer, l-inner) stream with a hardware SUB_DIM reset at page
    boundaries; the output AP has inner stride 0 so the last write per
    page (the full 6-term sum) lands at acc[p, m*64+c].  No separate
    multiplies/adds.
  - output written pixel-major [HW, (m,c)]; host reorders to NCHW.
"""

from contextlib import ExitStack

import numpy as np

N, C, H, W = 8, 64, 128, 128
INTER = 64
FEAT = 6
M = 6
KS = 3
L = KS * KS          # 9
NMID = 5
GOUT = FEAT * M      # 36
OUTC = C * M         # 384
HP, WP = H + 2, W + 2          # 130
NPAD = HP * WP                 # 16900
HWTOT = H * W                  # 16384
BN_EPS = 1e-5
NBAND = 32                     # 4-row bands
RPB = 4                        # rows per band
NT = RPB * W                   # 512 free elems per conv tile
NPAIR = NBAND // 2             # 16 band pairs

_CACHE = {}


def _register_seg_mac():
    """Register the segmented MAC-scan custom DVE op (idempotent)."""
    import concourse.dve_ops as dops

    for op in dops.OPS:
        if op.name == "ADAPT_SEG_MAC":
            return op
    from dataclasses import dataclass

    from concourse import dve_spec as ds
    from concourse.dve_spec import AluOp, Spec, Src0, Src1, scan
    from concourse.dve_table_gen import dve_ver_for
    from concourse.dve_uop import DveOpSpec

    def _ref(in0, in1, s0, s1, imm2):
        prod = in0.astype(np.float32) * in1.astype(np.float32)
        return np.cumsum(prod, axis=-1)

    spec = Spec(body=scan(AluOp.ADD, Src0 * Src1), reference=_ref)

    def _lower_seg(ver):
        assert ver == "v3"
        ds._validate_body(spec, ver)
        sp = ds._hoist_stream_invariant_ops(spec)
        scans = ds._collect(sp.body, ds.Scan)
        sc = scans[0]
        p = ds._build_placement(sp, scans, ds.N_STAGES[ver], ds.N_LANES[ver])
        d = p.node_stage[sc]
        seed_ov = {d: ds._node_as_stage(ds._scan_init(sc))}
        reset_ov = {d: ds._Stage(AluOp.BYPASS, sc.expr)}
        T = ds.Trigger
        states = [
            ds._State(placement=p, overrides=seed_ov, trigger=ds.COUNT_ONCE,
                      repeat=1, next=(1, 0, 0), write_out=False),
            ds._State(placement=p, consume=(True, True),
                      trigger=(T.SRC_TENSOR_DONE, T.SUB_DIM_DONE, T.NONE),
                      next=(0, 2, 0)),
            ds._State(placement=p, consume=(True, True), overrides=reset_ov,
                      trigger=(T.SRC_TENSOR_DONE, T.SUB_DIM_DONE, T.COUNT),
                      next=(0, 2, 1), repeat=1),
        ]
        uops = [ds._assemble(s) for s in states]
        for u in uops:
            u.validate(ver)
        return uops

    @dataclass(frozen=True)
    class SegDveOp(dops.DveOp):
        def compile(self, ver):
            key = (self.name, ver)
            c = dops._COMPILE_CACHE.get(key)
            if c is None:
                c = DveOpSpec(
                    name=self.name,
                    opcode=dops.get_dve_sub_opcode(self.name),
                    uops=_lower_seg(ver),
                    rd1_en=True,
                )
                dops._COMPILE_CACHE[key] = c
            return c

    ver = dve_ver_for("TRN2")
    row = dops._CUSTOM_DVE_ROW_BASE + len(dops.OPS)
    dops._SUB_OPCODE_FOR_NAME["ADAPT_SEG_MAC"] = row
    op = SegDveOp("ADAPT_SEG_MAC", spec, subdim=True, uops_sha={ver: "unused"})
    dops.OPS.append(op)
    dops.CUSTOM_DVE_SPECS[op.name] = spec
    return op


def _build_program():
    import concourse.bacc as bacc
    import concourse.mybir as mybir
    from concourse.tile import TileContext

    fp32 = mybir.dt.float32
    fp16 = mybir.dt.float16
    AF = mybir.ActivationFunctionType

    seg_op = _register_seg_mac()

    nc = bacc.Bacc("TRN2", debug=False)

    # ---------------- DRAM I/O ----------------
    x_d = nc.dram_tensor("x", [C, H, W], fp32, kind="ExternalInput")
    # paired stationaries, host layout [128 (=2 ky x 64 ic), 7*3*64]
    wpair_d = nc.dram_tensor("wpair", [128, 7 * 3 * 64], fp16, kind="ExternalInput")
    # single (ky=2) stationaries duplicated on both partition halves
    wsing_d = nc.dram_tensor("wsing", [128, 7 * 3 * 64], fp16, kind="ExternalInput")
    bias_d = nc.dram_tensor("bias", [64, 7], fp32, kind="ExternalInput")
    zwpair_d = nc.dram_tensor("zwpair", [128, 9 * 128], fp16, kind="ExternalInput")
    zwsing_d = nc.dram_tensor("zwsing", [128, 9 * 128], fp16, kind="ExternalInput")
    ident_d = nc.dram_tensor("ident", [128, 128], fp16, kind="ExternalInput")
    out_d = nc.dram_tensor("out", [HWTOT, OUTC], fp16, kind="ExternalOutput")

    with TileContext(nc) as tc, ExitStack() as es:
        # ------------- persistent SBUF -------------
        x2 = nc.alloc_sbuf_tensor("x2", [128, NPAD], fp16)
        t2a = nc.alloc_sbuf_tensor("t2a", [128, NPAD], fp16)
        t2b = nc.alloc_sbuf_tensor("t2b", [128, NPAD], fp16)
        wpair_sb = nc.alloc_sbuf_tensor("wpair_sb", [128, 7 * 3 * 64], fp16)
        wsing_sb = nc.alloc_sbuf_tensor("wsing_sb", [128, 7 * 3 * 64], fp16)
        bias_sb = nc.alloc_sbuf_tensor("bias_sb", [64, 7], fp32)
        zwpair_sb = nc.alloc_sbuf_tensor("zwpair_sb", [128, 9 * 128], fp16)
        zwsing_sb = nc.alloc_sbuf_tensor("zwsing_sb", [128, 9 * 128], fp16)
        ident_sb = nc.alloc_sbuf_tensor("ident_sb", [128, 128], fp16)
        gen_sb = nc.alloc_sbuf_tensor("gen_sb", [GOUT, HWTOT], fp16)

        x2v = x2[:].rearrange("p (h w) -> p h w", h=HP, w=WP)
        t2av = t2a[:].rearrange("p (h w) -> p h w", h=HP, w=WP)
        t2bv = t2b[:].rearrange("p (h w) -> p h w", h=HP, w=WP)

        # ------------- load constants -------------
        # Only the pad borders need zeroing (interiors are fully written):
        for bufv in (x2v, t2av, t2bv):
            nc.gpsimd.memset(bufv[:, 0:1, :], 0.0)       # padded row 0
            nc.gpsimd.memset(bufv[:, 129:130, :], 0.0)   # padded row 129
            nc.gpsimd.memset(bufv[64:128, 128:129, :], 0.0)  # bottom-half row 128
            nc.gpsimd.memset(bufv[:, :, 0:1], 0.0)       # left pad col
            nc.gpsimd.memset(bufv[:, :, 129:130], 0.0)   # right pad col
        # x (f32 -> f16): top half holds padded image at rows 1..128;
        # bottom half the same image one padded row up.
        for c0 in range(0, H, 16):
            nc.gpsimd.dma_start(out=x2v[0:64, 1 + c0:17 + c0, 1:129],
                                in_=x_d[:, c0:c0 + 16, :])
            nc.gpsimd.dma_start(out=x2v[64:128, c0:c0 + 16, 1:129],
                                in_=x_d[:, c0:c0 + 16, :])
        nc.sync.dma_start(out=wpair_sb[:], in_=wpair_d[:])
        nc.sync.dma_start(out=wsing_sb[:], in_=wsing_d[:])
        nc.sync.dma_start(out=bias_sb[:], in_=bias_d[:])
        nc.sync.dma_start(out=zwpair_sb[:], in_=zwpair_d[:])
        nc.sync.dma_start(out=zwsing_sb[:], in_=zwsing_d[:])
        nc.sync.dma_start(out=ident_sb[:], in_=ident_d[:])

        # ------------- tile pools -------------
        conv_ps = es.enter_context(tc.tile_pool(name="conv_ps", bufs=2, space="PSUM"))
        zb_ps_pool = es.enter_context(tc.tile_pool(name="zb_ps", bufs=3, space="PSUM"))
        tr_ps_pool = es.enter_context(tc.tile_pool(name="tr_ps", bufs=2, space="PSUM"))
        zb_pool = es.enter_context(tc.tile_pool(name="zb", bufs=6))
        et_pool = es.enter_context(tc.tile_pool(name="et", bufs=6))
        acc_pool = es.enter_context(tc.tile_pool(name="acc", bufs=6))

        layer_src = [x2v, t2av, t2bv, t2av, t2bv, t2av, t2bv]
        layer_dst = [t2av, t2bv, t2av, t2bv, t2av, t2bv, None]

        def conv_pair(lyr, pr):
            """One conv layer for band pair (2*pr, 2*pr+1), col-tiled: band A
            outputs to array cols 0-63 / PSUM partitions 0-63, band B to cols
            64-127 / partitions 64-127.  Single (ky=2) taps for band B read
            the shifted bottom-half buffer so they row-tile to (64,64)."""
            src = layer_src[lyr]
            bA, bB = 2 * pr, 2 * pr + 1
            rA, rB = bA * RPB, bB * RPB
            ps = conv_ps.tile([128, NT], fp32, tag="conv", name=f"cps_{lyr}_{pr}")
            psv = ps[:].rearrange("p (h w) -> p h w", h=RPB, w=W)
            psA, psB = psv[0:64], psv[64:128]
            for kx in range(3):
                off = (lyr * 3 + kx) * 64
                nc.tensor.matmul(psA, wpair_sb[:, off:off + 64],
                                 src[:, rA:rA + RPB, kx:kx + W],
                                 start=(kx == 0), stop=False)
                nc.tensor.matmul(psB, wpair_sb[:, off:off + 64],
                                 src[:, rB:rB + RPB, kx:kx + W],
                                 start=(kx == 0), stop=False)
            for kx in range(3):
                off = (lyr * 3 + kx) * 64
                nc.tensor.matmul(psA, wsing_sb[0:64, off:off + 64],
                                 src[0:64, rA + 2:rA + 2 + RPB, kx:kx + W],
                                 start=False, stop=(kx == 2))
                nc.tensor.matmul(psB, wsing_sb[64:128, off:off + 64],
                                 src[64:128, rB + 1:rB + 1 + RPB, kx:kx + W],
                                 start=False, stop=(kx == 2))
            if lyr < 6:
                dst = layer_dst[lyr]
                func = AF.Tanh if lyr == 0 else AF.Identity
                for half, r0 in ((psA, rA), (psB, rB)):
                    top = dst[0:64, r0 + 1:r0 + 1 + RPB, 1:1 + W]
                    nc.scalar.activation(top, half, func,
                                         bias=bias_sb[:, lyr:lyr + 1], scale=1.0)
                    bot = dst[64:128, r0:r0 + RPB, 1:1 + W]
                    nc.sync.dma_start(out=bot, in_=top)
            else:
                nc.scalar.activation(gen_sb[0:GOUT, bA * NT:(bA + 1) * NT],
                                     ps[0:GOUT, :], AF.Tanh,
                                     bias=bias_sb[0:GOUT, 6:7], scale=1.0)
                nc.scalar.activation(gen_sb[0:GOUT, bB * NT:(bB + 1) * NT],
                                     ps[64:64 + GOUT, :], AF.Tanh,
                                     bias=bias_sb[0:GOUT, 6:7], scale=1.0)

        zb_tiles = {}

        def zb_pair(pr):
            """6 fixed basis depthwise convs of x for band pair (2pr, 2pr+1),
            channel layout (l,c) in 3 col-groups of 128.  K=64 single taps of
            the two bands row-tile ((0,0) vs (64,0))."""
            bA, bB = 2 * pr, 2 * pr + 1
            rA, rB = bA * RPB, bB * RPB
            for g in range(3):
                tiles = []
                for band, r0 in ((bA, rA), (bB, rB)):
                    ps = zb_ps_pool.tile([128, NT], fp32, tag="zbps",
                                         name=f"zbps_{band}_{g}")
                    pv = ps[:].rearrange("p (h w) -> p h w", h=RPB, w=W)
                    tiles.append((band, r0, ps, pv))
                for kx in range(3):
                    off = (g * 3 + kx) * 128
                    for band, r0, ps, pv in tiles:
                        nc.tensor.matmul(pv, zwpair_sb[:, off:off + 128],
                                         x2v[:, r0:r0 + RPB, kx:kx + W],
                                         start=(kx == 0), stop=False)
                for kx in range(3):
                    off = (g * 3 + kx) * 128
                    band, r0, ps, pv = tiles[0]
                    nc.tensor.matmul(pv, zwsing_sb[0:64, off:off + 128],
                                     x2v[0:64, r0 + 2:r0 + 2 + RPB, kx:kx + W],
                                     start=False, stop=(kx == 2))
                    band, r0, ps, pv = tiles[1]
                    nc.tensor.matmul(pv, zwsing_sb[64:128, off:off + 128],
                                     x2v[64:128, r0 + 1:r0 + 1 + RPB, kx:kx + W],
                                     start=False, stop=(kx == 2))
                for band, r0, ps, pv in tiles:
                    t = zb_pool.tile([128, NT], fp16, tag=f"zb{g}",
                                     name=f"zb_{band}_{g}")
                    nc.scalar.activation(t[:], ps[:], AF.Copy)
                    zb_tiles[(band, g)] = t

        def row_einsum2(r):
            band, sub = r // RPB, r % RPB
            # all 4 transposes into one PSUM tile: [0:384]=zbT, [384:420]=genT
            tp = tr_ps_pool.tile([128, 420], fp16, tag="trps", name=f"tp_{r}")
            for g in range(3):
                nc.tensor.transpose(
                    tp[:, g * 128:(g + 1) * 128],
                    zb_tiles[(band, g)][:, sub * W:(sub + 1) * W],
                    ident_sb[:])
            nc.tensor.transpose(tp[:, 384:420], gen_sb[:, r * W:(r + 1) * W],
                                ident_sb[0:GOUT, 0:GOUT])
            et = et_pool.tile([128, 420], fp16, tag="et", name=f"et_{r}")
            nc.scalar.activation(et[:], tp[:], AF.Copy)

            acc = acc_pool.tile([128, OUTC], fp16, tag="acc")
            zt3 = et[:, 0:384].rearrange("p (l c) -> p c l", l=M)
            for m in range(M):
                in1 = (et[:, 384 + m * M:384 + (m + 1) * M]
                       .unsqueeze(1).to_broadcast((128, 64, M)))
                outv = (acc[:, m * 64:(m + 1) * 64]
                        .unsqueeze(2).to_broadcast((128, 64, M)))
                nc.vector._custom_dve(seg_op, out=outv, in0=zt3, in1=in1)
            nc.sync.dma_start(out=out_d[r * W:(r + 1) * W, :], in_=acc[:])

        # ------------- emission -------------
        # Ramp: depth-first conv for the first two pairs so the DVE starts
        # within ~15us, then breadth-wavefront conv for the rest (keeps the
        # PE dense and HAM-warm), with einsum2 pairs interleaved at the DVE's
        # pace.  zb is computed just-in-time before each pair's einsum2 rows
        # (it only needs x2), so no zb SBUF tiles persist beyond 2 pairs.
        HEAD = 2
        done = set()

        def conv_once(lyr, pr):
            if (lyr, pr) not in done:
                done.add((lyr, pr))
                conv_pair(lyr, pr)

        for pr in range(HEAD):
            zb_pair(pr)
        # Halo-correct dependency cone for gen of pair 0 first (einsum2 can
        # start earliest), then pair 1.  The 3x3 halo grows one row per
        # layer, but conv granularity is whole pairs, so for gen of pairs
        # 0..P, layer k must cover pairs 0..P+(6-k).
        for lyr in range(7):
            for pr in range(min(NPAIR, 7 - lyr)):
                conv_once(lyr, pr)
        for r in range(0, 2 * RPB):
            row_einsum2(r)
        for lyr in range(7):
            for pr in range(min(NPAIR, 8 - lyr)):
                conv_once(lyr, pr)
        for r in range(2 * RPB, 2 * 2 * RPB):
            row_einsum2(r)
        # remaining conv as a diagonal wavefront (dense, dependency-clean)
        for step in range(NPAIR + 6):
            for lyr in range(7):
                pr = step - lyr
                if 0 <= pr < NPAIR:
                    conv_once(lyr, pr)
        # einsum2 for the rest, with zb prefetched one pair ahead so the
        # (cold) zb matmuls + copies hide behind the previous pair's scans.
        zb_pair(HEAD)
        for pe in range(HEAD, NPAIR):
            if pe + 1 < NPAIR:
                zb_pair(pe + 1)
            for r in range(pe * 2 * RPB, (pe + 1) * 2 * RPB):
                row_einsum2(r)

    nc.finalize()
    return nc


def _prep_inputs(inputs):
    """Host-side weight prep: BN folding, tap pairing, basis stationaries."""
    bf = np.float16

    f = lambda k: np.asarray(inputs[k], np.float32)
    W0, b0, g0, be0, m0, v0 = (f(k) for k in ("W0", "b0", "g0", "be0", "m0", "v0"))
    Wmid, bmid = f("Wmid"), f("bmid")
    Wf, bf_, gf, bef, mf, vf = (f(k) for k in ("Wf", "bf", "gf", "bef", "mf", "vf"))
    bases = f("bases")

    s0 = g0 / np.sqrt(v0 + BN_EPS)
    W0p = W0 * s0[:, None, None, None]
    b0p = (b0 - m0) * s0 + be0
    sf = gf / np.sqrt(vf + BN_EPS)
    Wfp = Wf * sf[:, None, None, None]
    bfp = (bf_ - mf) * sf + bef

    # layer weights [oc, ic, ky, kx] -> paired/single stationaries
    Wf64 = np.zeros((64, 64, 3, 3), np.float32)
    Wf64[:GOUT] = Wfp
    Ws = [W0p] + [Wmid[i] for i in range(NMID)] + [Wf64]
    wpair = np.zeros((7, 3, 128, 64), np.float32)
    wsing = np.zeros((7, 3, 64, 64), np.float32)
    for lyr in range(7):
        w = Ws[lyr]
        for kx in range(3):
            wpair[lyr, kx, 0:64] = w[:, :, 0, kx].T     # ky=0 -> top partitions
            wpair[lyr, kx, 64:128] = w[:, :, 1, kx].T   # ky=1 -> bottom
            wsing[lyr, kx] = w[:, :, 2, kx].T           # ky=2

    bias = np.zeros((64, 7), np.float32)
    bias[:, 0] = b0p
    for i in range(NMID):
        bias[:, 1 + i] = bmid[i]
    bias[:GOUT, 6] = bfp

    # zb (basis depthwise conv) stationaries: col-group g holds channels
    # (l, c) for l in {2g, 2g+1}; value = bases[l, ky*3+kx] on the diagonal.
    zwpair = np.zeros((3, 3, 128, 128), np.float32)
    zwsing = np.zeros((3, 3, 64, 128), np.float32)
    eye = np.eye(64, dtype=np.float32)
    for g in range(3):
        for kx in range(3):
            for lh in range(2):
                l = 2 * g + lh
                for ky in range(2):
                    zwpair[g, kx, ky * 64:(ky + 1) * 64, lh * 64:(lh + 1) * 64] = \
                        eye * bases[l, ky * 3 + kx]
                zwsing[g, kx, :, lh * 64:(lh + 1) * 64] = eye * bases[l, 6 + kx]
    zwpair = np.ascontiguousarray(zwpair.transpose(2, 0, 1, 3)).reshape(128, 9 * 128)
    zwsing = np.ascontiguousarray(zwsing.transpose(2, 0, 1, 3)).reshape(64, 9 * 128)
    zwsing2 = np.concatenate([zwsing, zwsing], axis=0)  # duplicate on both halves

    ident = np.eye(128, dtype=np.float32)
    wpair = wpair.transpose(2, 0, 1, 3).reshape(128, 7 * 3 * 64)
    wsing = wsing.transpose(2, 0, 1, 3).reshape(64, 7 * 3 * 64)
    wsing2 = np.concatenate([wsing, wsing], axis=0)
    return {
        "wpair": np.ascontiguousarray(wpair).astype(bf),
        "wsing": np.ascontiguousarray(wsing2).astype(bf),
        "bias": bias,
        "zwpair": zwpair.astype(bf),
        "zwsing": zwsing2.astype(bf),
        "ident": ident.astype(bf),
    }


def _env_int(name, default):
    import os
    v = os.environ.get(name)
    return default if v is None else int(v)


def _install_ntff_hook():
    """Provide antenv.axon_hooks (missing in this image) so bass_utils can
    NTFF-profile under axon via the injected libaxon_pjrt.so."""
    import sys
    import types
    if "antenv.axon_hooks" in sys.modules:
        return
    try:
        import antenv
        from trn_agent_boot.trn_boot import _ntff_profile_via_ctypes
        hook = _ntff_profile_via_ctypes("/opt/axon/libaxon_pjrt.so")
    except Exception:
        return
    mod = types.ModuleType("antenv.axon_hooks")
    holder = {"h": hook}
    mod.set_axon_ntff_profile_hook = lambda h: holder.__setitem__("h", h)
    mod.get_axon_ntff_profile_hook = lambda: holder.get("h")
    sys.modules["antenv.axon_hooks"] = mod
    antenv.axon_hooks = mod


def kernel(**inputs):
    from concourse import bass_utils

    if "prog" not in _CACHE:
        _CACHE["prog"] = _build_program()
    nc = _CACHE["prog"]

    shared = _prep_inputs(inputs)
    x_full = np.asarray(inputs["input"], np.float32)
    in_maps = [dict(shared, x=np.ascontiguousarray(x_full[i])) for i in range(N)]

    trace = bool(_env_int("ADAPT_TRACE", 0))
    if trace:
        _install_ntff_hook()
    res = bass_utils.run_bass_kernel_spmd(
        nc, in_maps, core_ids=list(range(N)), trace=trace)
    if trace:
        _CACHE["last_result"] = res

    out = np.empty((N, OUTC, H, W), np.float32)
    for i in range(N):
        o = np.asarray(res.results[i]["out"], dtype=np.float32)  # [HW, (m,c)]
        o4 = o.reshape(H, W, M, C)
        out[i] = o4.transpose(3, 2, 0, 1).reshape(OUTC, H, W)
    return out


if __name__ == "__main__":
    import time
    t0 = time.time()
    nc = _build_program()
    print(f"program built in {time.time() - t0:.1f}s")
